# revision 4
# baseline (speedup 1.0000x reference)
"""VQ codebook-lookup kernel for trn2 (8 NeuronCores, SPMD data-parallel).

Computes, for x: [32, 64, 64, 64] (BCHW) and codebook: [1024, 64]:
    flat = BHWC-flattened x                       # [N, 64]
    d = ||flat||^2 + ||e||^2 - 2 flat @ e^T       # [N, 1024], f32 rounding
    out = e[argmin d] in BCHW layout.

The argmin must match the f32 reference bit-for-bit on near-ties, so the
kernel replicates the reference's rounding structure:
    nd = fl(c - fl(A+b)),  c = flat @ (2e)^T  (accurate, small magnitude)
with A = ||flat||^2 (host, f32), b = ||e||^2 (host, f32). The fl(A+b) inner
rounding is reproduced on the PE via a 6-row bf16 chain (exact 3-way bf16
splits of A and b; the PE accumulates a matmul chain wide and rounds once
on the PSUM write). c is accumulated first at small magnitude via bf16x2
split matmuls, so the final merge is the single f32 rounding fl(c - T).

Sharding: batch-parallel. Core i handles batches [4i, 4i+4), processed as
2 pairs of 2 batches (the pair shares a [128, 4096] SBUF tile; contraction
runs on partition strips 0:64 / 64:128 as concurrent row-tiled matmuls).
"""

import sys
import numpy as np
import ml_dtypes
from contextlib import ExitStack

for p in ("/opt/trn_rl_repo",):
    if p not in sys.path:
        sys.path.append(p)

import concourse.bacc as bacc
import concourse.mybir as mybir
import concourse.tile as tile
from concourse import bass_utils, library_config

F32 = mybir.dt.float32
BF16 = mybir.dt.bfloat16
U32 = mybir.dt.uint32
I16 = mybir.dt.int16

B, D, H, W = 32, 64, 64, 64
K = 1024
NCORES = 8
BPC = B // NCORES          # batches per core = 4
TOK = H * W                # tokens per batch = 4096
NTILE = TOK // 128         # 128-token tiles per batch = 32

_cache = {}


def _bf16(v):
    return v.astype(ml_dtypes.bfloat16)


def _split2(v):
    h = _bf16(v)
    l = _bf16(v - h.astype(np.float32))
    return h, l


def _split3_neg(v):
    """exact 3-way bf16 split of -v (bf16 h1+h2+h3 == -v exactly for normals)"""
    v = -v.astype(np.float32)
    h1 = _bf16(v)
    r = v - h1.astype(np.float32)
    h2 = _bf16(r)
    h3 = _bf16(r - h2.astype(np.float32))
    return h1, h2, h3


def _build_module():
    nc = bacc.Bacc("TRN2", target_bir_lowering=False, debug=False, num_devices=NCORES)

    d_xh = nc.dram_tensor("xh", [2, 128, TOK], BF16, kind="ExternalInput").ap()
    d_xl = nc.dram_tensor("xl", [2, 128, TOK], BF16, kind="ExternalInput").ap()
    d_e2h = nc.dram_tensor("e2h", [128, K], BF16, kind="ExternalInput").ap()
    d_e2l = nc.dram_tensor("e2l", [128, K], BF16, kind="ExternalInput").ap()
    d_tml = nc.dram_tensor("tml", [2, 12, TOK], BF16, kind="ExternalInput").ap()
    d_tmr = nc.dram_tensor("tmr", [12, K], BF16, kind="ExternalInput").ap()
    d_cbt = nc.dram_tensor("cbt", [128, K], F32, kind="ExternalInput").ap()
    d_xf = nc.dram_tensor("xf", [2, 128, TOK], F32, kind="ExternalInput").ap()
    d_out = nc.dram_tensor("out", [2, 128, TOK], F32, kind="ExternalOutput").ap()

    with tile.TileContext(nc) as tc, ExitStack() as ctx:
        sb = ctx.enter_context(tc.tile_pool(name="sb", bufs=1))
        sb2 = ctx.enter_context(tc.tile_pool(name="sb2", bufs=2))
        sb3 = ctx.enter_context(tc.tile_pool(name="sb3", bufs=3))
        ps = ctx.enter_context(tc.tile_pool(name="ps", bufs=2, space="PSUM"))
        dr = ctx.enter_context(tc.tile_pool(name="dr", bufs=2, space="DRAM"))

        nc.gpsimd.load_library(library_config.ap_gather)

        # loop-invariant operands
        e2ht = sb.tile([128, K], BF16, tag="e2ht")
        nc.sync.dma_start(e2ht[:], d_e2h[:])
        e2lt = sb.tile([128, K], BF16, tag="e2lt")
        nc.sync.dma_start(e2lt[:], d_e2l[:])
        tmr = sb.tile([128, K], BF16, tag="tmr")
        nc.sync.dma_start(tmr[0:6, :], d_tmr[0:6, :])
        nc.sync.dma_start(tmr[64:70, :], d_tmr[6:12, :])
        cbt = sb.tile([128, K], F32, tag="cbt")
        nc.sync.dma_start(cbt[:], d_cbt[:])

        for p in range(2):
            xht = sb2.tile([128, TOK], BF16, tag="xh")
            nc.sync.dma_start(xht[:], d_xh[p])
            xlt = sb2.tile([128, TOK], BF16, tag="xl")
            nc.sync.dma_start(xlt[:], d_xl[p])
            xft = sb2.tile([128, TOK], F32, tag="xf")
            nc.sync.dma_start(xft[:], d_xf[p])
            tml = sb2.tile([128, TOK], BF16, tag="tml")
            nc.sync.dma_start(tml[0:6, :], d_tml[p, 0:6, :])
            nc.sync.dma_start(tml[64:70, :], d_tml[p, 6:12, :])

            idxc = [sb2.tile([128, NTILE], I16, tag=f"idxc{h}", name=f"idxc{h}_{p}")
                    for h in range(2)]

            for g in range(NTILE):
                gs = slice(g * 128, (g + 1) * 128)
                pst = [ps.tile([128, K], F32, tag="psA", name=f"psA_{p}_{g}"),
                       ps.tile([128, K], F32, tag="psB", name=f"psB_{p}_{g}")]
                for ch in range(2):
                    cs = slice(ch * 512, (ch + 1) * 512)
                    for h, lo, hi in ((0, 0, 64), (1, 64, 128)):
                        pp = pst[h][:, cs]
                        nc.tensor.matmul(pp, xht[lo:hi, gs], e2ht[lo:hi, cs],
                                         start=True, stop=False)
                        nc.tensor.matmul(pp, xlt[lo:hi, gs], e2ht[lo:hi, cs],
                                         start=False, stop=False)
                        nc.tensor.matmul(pp, xht[lo:hi, gs], e2lt[lo:hi, cs],
                                         start=False, stop=False)
                        nc.tensor.matmul(pp, tml[lo:lo + 6, gs], tmr[lo:lo + 6, cs],
                                         start=False, stop=True)
                for h in range(2):
                    nd = sb3.tile([128, K], F32, tag=f"nd{h}")
                    nc.scalar.copy(nd[:], pst[h][:])
                    mx8 = sb3.tile([128, 8], F32, tag=f"mx{h}")
                    nc.vector.max(mx8[:], nd[:])
                    ix8 = sb3.tile([128, 8], mybir.dt.uint16, tag=f"ix{h}")
                    nc.vector.max_index(ix8[:], mx8[:], nd[:])
                    nc.vector.tensor_copy(idxc[h][:, g:g + 1],
                                          ix8[:, 0:1].bitcast(I16))

            # stage indices to DRAM, re-read in ap_gather wrapped layout
            agx = sb2.tile([128, 256], I16, tag="agx")
            for h in range(2):
                st = dr.tile([128, NTILE], I16, tag=f"st{h}")
                nc.sync.dma_start(st[:], idxc[h][:])
                src = st[:].rearrange("(b r) g -> r g b", b=8, r=16)
                for c in range(4):
                    q = 16 * (4 * h + c)
                    dst = agx[q:q + 16, :].rearrange("p (a b) -> p a b", a=32, b=8)
                    nc.sync.dma_start(dst, src)

            gout = sb2.tile([128, TOK], F32, tag="gout")
            nc.gpsimd.ap_gather(gout[:], cbt[:], agx[:],
                                channels=128, num_elems=K, d=1, num_idxs=TOK)
            # straight-through estimator rounding: out = fl(x + fl(q - x))
            nc.vector.tensor_tensor(gout[:], gout[:], xft[:],
                                    mybir.AluOpType.subtract)
            nc.vector.tensor_tensor(gout[:], gout[:], xft[:],
                                    mybir.AluOpType.add)
            nc.sync.dma_start(d_out[p], gout[:])

    nc.compile()
    return nc


def _prep_host(inputs, codebook):
    x = np.ascontiguousarray(inputs, dtype=np.float32)
    cb = np.ascontiguousarray(codebook, dtype=np.float32)

    # A = ||flat||^2 with the reference's summation (contiguous last-axis np.sum)
    flat = np.ascontiguousarray(x.transpose(0, 2, 3, 1)).reshape(-1, D)
    A = np.sum(flat * flat, axis=1)              # f32 [N]
    A = A.reshape(B, TOK)
    b = np.sum(cb * cb, axis=1)                  # f32 [K]

    xh, xl = _split2(x)                          # BCHW layout == [b, 64, 4096] channel-major
    xh = xh.reshape(B, 128 // 2, TOK)            # keep [B, 64, TOK]
    xl = xl.reshape(B, 128 // 2, TOK)

    e2 = (2.0 * cb).astype(np.float32)           # exact
    e2h, e2l = _split2(e2.T)                     # [64, 1024] each
    e2h_d = np.concatenate([e2h, e2h], axis=0)   # [128, K]
    e2l_d = np.concatenate([e2l, e2l], axis=0)

    nb1, nb2, nb3 = _split3_neg(b)               # -b splits, [K] bf16
    ones_k = np.ones(K, ml_dtypes.bfloat16)
    tmr = np.stack([nb1, nb2, nb3, ones_k, ones_k, ones_k] * 2, axis=0)  # [12, K]

    nA1, nA2, nA3 = _split3_neg(A)               # [B, TOK] bf16 each
    ones_t = np.ones(TOK, ml_dtypes.bfloat16)

    cbt = np.ascontiguousarray(cb.T)             # [64, K]
    cbt_d = np.concatenate([cbt, cbt], axis=0)   # [128, K]

    in_maps = []
    for cid in range(NCORES):
        b0 = BPC * cid
        xh_c = xh[b0:b0 + 4].reshape(2, 128, TOK)
        xl_c = xl[b0:b0 + 4].reshape(2, 128, TOK)
        tml = np.empty((2, 12, TOK), ml_dtypes.bfloat16)
        for p in range(2):
            bA, bB = b0 + 2 * p, b0 + 2 * p + 1
            for r in range(3):
                tml[p, r] = ones_t
                tml[p, 6 + r] = ones_t
            tml[p, 3], tml[p, 4], tml[p, 5] = nA1[bA], nA2[bA], nA3[bA]
            tml[p, 9], tml[p, 10], tml[p, 11] = nA1[bB], nA2[bB], nA3[bB]
        in_maps.append({
            "xf": np.ascontiguousarray(x[b0:b0 + 4].reshape(2, 128, TOK)),
            "xh": np.ascontiguousarray(xh_c),
            "xl": np.ascontiguousarray(xl_c),
            "e2h": e2h_d, "e2l": e2l_d,
            "tml": tml, "tmr": tmr,
            "cbt": cbt_d,
        })
    return in_maps


def _run(inputs, codebook, trace=False):
    if "nc" not in _cache:
        _cache["nc"] = _build_module()
    nc = _cache["nc"]
    in_maps = _prep_host(inputs, codebook)
    res = bass_utils.run_bass_kernel_spmd(
        nc, in_maps, core_ids=list(range(NCORES)), trace=trace)
    outs = np.empty((B, D, H, W), np.float32)
    for cid in range(NCORES):
        o = res.results[cid]["out"]              # [2, 128, TOK]
        outs[BPC * cid: BPC * (cid + 1)] = o.reshape(BPC, D, H, W)
    return outs, res


def kernel(inputs, codebook):
    out, _ = _run(inputs, codebook, trace=False)
    return out


# revision 5
# speedup vs baseline: 6479.2485x; 6479.2485x over previous
"""VQ codebook-lookup kernel for trn2 (8 NeuronCores, SPMD data-parallel).

Computes, for x: [32, 64, 64, 64] (BCHW) and codebook: [1024, 64]:
    flat = BHWC-flattened x                       # [N, 64]
    d = ||flat||^2 + ||e||^2 - 2 flat @ e^T       # [N, 1024], f32 rounding
    out = e[argmin d] in BCHW layout.

The argmin must match the f32 reference bit-for-bit on near-ties, so the
kernel replicates the reference's rounding structure:
    nd = fl(c - fl(A+b)),  c = flat @ (2e)^T  (accurate, small magnitude)
with A = ||flat||^2 (host, f32), b = ||e||^2 (host, f32). The fl(A+b) inner
rounding is reproduced on the PE via a 6-row bf16 chain (exact 3-way bf16
splits of A and b; the PE accumulates a matmul chain wide and rounds once
on the PSUM write). c is accumulated first at small magnitude via bf16x2
split matmuls, so the final merge is the single f32 rounding fl(c - T).

Sharding: batch-parallel. Core i handles batches [4i, 4i+4), processed as
2 pairs of 2 batches (the pair shares a [128, 4096] SBUF tile; contraction
runs on partition strips 0:64 / 64:128 as concurrent row-tiled matmuls).
"""

import sys
import numpy as np
import ml_dtypes
from contextlib import ExitStack

for p in ("/opt/trn_rl_repo",):
    if p not in sys.path:
        sys.path.append(p)

import concourse.bacc as bacc
import concourse.mybir as mybir
import concourse.tile as tile
from concourse import bass_utils, library_config

F32 = mybir.dt.float32
BF16 = mybir.dt.bfloat16
U32 = mybir.dt.uint32
I16 = mybir.dt.int16

B, D, H, W = 32, 64, 64, 64
K = 1024
NCORES = 8
BPC = B // NCORES          # batches per core = 4
TOK = H * W                # tokens per batch = 4096
NTILE = TOK // 128         # 128-token tiles per batch = 32

_cache = {}


def _bf16(v):
    return v.astype(ml_dtypes.bfloat16)


def _split2(v):
    h = _bf16(v)
    l = _bf16(v - h.astype(np.float32))
    return h, l


def _split3_neg(v):
    """exact 3-way bf16 split of -v (bf16 h1+h2+h3 == -v exactly for normals)"""
    v = -v.astype(np.float32)
    h1 = _bf16(v)
    r = v - h1.astype(np.float32)
    h2 = _bf16(r)
    h3 = _bf16(r - h2.astype(np.float32))
    return h1, h2, h3


def _build_module():
    nc = bacc.Bacc("TRN2", target_bir_lowering=False, debug=False, num_devices=NCORES)

    d_xh = nc.dram_tensor("xh", [2, 128, TOK], BF16, kind="ExternalInput").ap()
    d_xl = nc.dram_tensor("xl", [2, 128, TOK], BF16, kind="ExternalInput").ap()
    d_e2h = nc.dram_tensor("e2h", [128, K], BF16, kind="ExternalInput").ap()
    d_e2l = nc.dram_tensor("e2l", [128, K], BF16, kind="ExternalInput").ap()
    d_tml = nc.dram_tensor("tml", [2, 12, TOK], BF16, kind="ExternalInput").ap()
    d_tmr = nc.dram_tensor("tmr", [12, K], BF16, kind="ExternalInput").ap()
    d_cbt = nc.dram_tensor("cbt", [128, K], F32, kind="ExternalInput").ap()
    d_xf = nc.dram_tensor("xf", [2, 128, TOK], F32, kind="ExternalInput").ap()
    d_out = nc.dram_tensor("out", [2, 128, TOK], F32, kind="ExternalOutput").ap()

    with tile.TileContext(nc) as tc, ExitStack() as ctx:
        sb = ctx.enter_context(tc.tile_pool(name="sb", bufs=1))
        sb2 = ctx.enter_context(tc.tile_pool(name="sb2", bufs=2))
        sb3 = ctx.enter_context(tc.tile_pool(name="sb3", bufs=3))
        ps = ctx.enter_context(tc.tile_pool(name="ps", bufs=2, space="PSUM"))
        dr = ctx.enter_context(tc.tile_pool(name="dr", bufs=2, space="DRAM"))

        nc.gpsimd.load_library(library_config.ap_gather)

        # loop-invariant operands
        e2ht = sb.tile([128, K], BF16, tag="e2ht")
        nc.sync.dma_start(e2ht[:], d_e2h[:])
        e2lt = sb.tile([128, K], BF16, tag="e2lt")
        nc.sync.dma_start(e2lt[:], d_e2l[:])
        tmr = sb.tile([128, K], BF16, tag="tmr")
        nc.sync.dma_start(tmr[0:6, :], d_tmr[0:6, :])
        nc.sync.dma_start(tmr[64:70, :], d_tmr[6:12, :])
        cbt = sb.tile([128, K], F32, tag="cbt")
        nc.sync.dma_start(cbt[:], d_cbt[:])

        for p in range(2):
            xht = sb2.tile([128, TOK], BF16, tag="xh")
            nc.sync.dma_start(xht[:], d_xh[p])
            xlt = sb2.tile([128, TOK], BF16, tag="xl")
            nc.sync.dma_start(xlt[:], d_xl[p])
            xft = sb2.tile([128, TOK], F32, tag="xf")
            nc.sync.dma_start(xft[:], d_xf[p])
            tml = sb2.tile([128, TOK], BF16, tag="tml")
            nc.sync.dma_start(tml[0:6, :], d_tml[p, 0:6, :])
            nc.sync.dma_start(tml[64:70, :], d_tml[p, 6:12, :])

            idxc = [sb2.tile([128, NTILE * 8], mybir.dt.uint16, tag=f"idxc{h}",
                             name=f"idxc{h}_{p}") for h in range(2)]

            for g in range(NTILE):
                gs = slice(g * 128, (g + 1) * 128)
                pst = [ps.tile([128, K], F32, tag="psA", name=f"psA_{p}_{g}"),
                       ps.tile([128, K], F32, tag="psB", name=f"psB_{p}_{g}")]
                for ch in range(2):
                    cs = slice(ch * 512, (ch + 1) * 512)
                    for h, lo, hi in ((0, 0, 64), (1, 64, 128)):
                        pp = pst[h][:, cs]
                        nc.tensor.matmul(pp, xht[lo:hi, gs], e2ht[lo:hi, cs],
                                         start=True, stop=False)
                        nc.tensor.matmul(pp, xlt[lo:hi, gs], e2ht[lo:hi, cs],
                                         start=False, stop=False)
                        nc.tensor.matmul(pp, xht[lo:hi, gs], e2lt[lo:hi, cs],
                                         start=False, stop=False)
                        nc.tensor.matmul(pp, tml[lo:lo + 6, gs], tmr[lo:lo + 6, cs],
                                         start=False, stop=True)
                for h in range(2):
                    nd = sb3.tile([128, K], F32, tag=f"nd{h}")
                    nc.scalar.copy(nd[:], pst[h][:])
                    mx8 = sb3.tile([128, 8], F32, tag=f"mx{h}")
                    nc.vector.max(mx8[:], nd[:])
                    nc.vector.max_index(idxc[h][:, g * 8:(g + 1) * 8],
                                        mx8[:], nd[:])

            # stage indices to DRAM, re-read in ap_gather wrapped layout
            agx = sb2.tile([128, 256], I16, tag="agx")
            for h in range(2):
                st = dr.tile([128, NTILE], I16, tag=f"st{h}")
                nc.sync.dma_start(
                    st[:], idxc[h][:].bitcast(I16).rearrange(
                        "p (g e) -> p g e", e=8)[:, :, 0])
                src = st[:].rearrange("(b r) g -> r g b", b=8, r=16)
                for c in range(4):
                    q = 16 * (4 * h + c)
                    dst = agx[q:q + 16, :].rearrange("p (a b) -> p a b", a=32, b=8)
                    nc.sync.dma_start(dst, src)

            gout = sb2.tile([128, TOK], F32, tag="gout")
            nc.gpsimd.ap_gather(gout[:], cbt[:], agx[:],
                                channels=128, num_elems=K, d=1, num_idxs=TOK)
            # straight-through estimator rounding: out = fl(x + fl(q - x))
            nc.vector.tensor_tensor(gout[:], gout[:], xft[:],
                                    mybir.AluOpType.subtract)
            nc.vector.tensor_tensor(gout[:], gout[:], xft[:],
                                    mybir.AluOpType.add)
            nc.sync.dma_start(d_out[p], gout[:])

    nc.compile()
    return nc


def _prep_host(inputs, codebook):
    x = np.ascontiguousarray(inputs, dtype=np.float32)
    cb = np.ascontiguousarray(codebook, dtype=np.float32)

    # A = ||flat||^2 with the reference's summation (contiguous last-axis np.sum)
    flat = np.ascontiguousarray(x.transpose(0, 2, 3, 1)).reshape(-1, D)
    A = np.sum(flat * flat, axis=1)              # f32 [N]
    A = A.reshape(B, TOK)
    b = np.sum(cb * cb, axis=1)                  # f32 [K]

    xh, xl = _split2(x)                          # BCHW layout == [b, 64, 4096] channel-major
    xh = xh.reshape(B, 128 // 2, TOK)            # keep [B, 64, TOK]
    xl = xl.reshape(B, 128 // 2, TOK)

    e2 = (2.0 * cb).astype(np.float32)           # exact
    e2h, e2l = _split2(e2.T)                     # [64, 1024] each
    e2h_d = np.concatenate([e2h, e2h], axis=0)   # [128, K]
    e2l_d = np.concatenate([e2l, e2l], axis=0)

    nb1, nb2, nb3 = _split3_neg(b)               # -b splits, [K] bf16
    ones_k = np.ones(K, ml_dtypes.bfloat16)
    tmr = np.stack([nb1, nb2, nb3, ones_k, ones_k, ones_k] * 2, axis=0)  # [12, K]

    nA1, nA2, nA3 = _split3_neg(A)               # [B, TOK] bf16 each
    ones_t = np.ones(TOK, ml_dtypes.bfloat16)

    cbt = np.ascontiguousarray(cb.T)             # [64, K]
    cbt_d = np.concatenate([cbt, cbt], axis=0)   # [128, K]

    in_maps = []
    for cid in range(NCORES):
        b0 = BPC * cid
        xh_c = xh[b0:b0 + 4].reshape(2, 128, TOK)
        xl_c = xl[b0:b0 + 4].reshape(2, 128, TOK)
        tml = np.empty((2, 12, TOK), ml_dtypes.bfloat16)
        for p in range(2):
            bA, bB = b0 + 2 * p, b0 + 2 * p + 1
            for r in range(3):
                tml[p, r] = ones_t
                tml[p, 6 + r] = ones_t
            tml[p, 3], tml[p, 4], tml[p, 5] = nA1[bA], nA2[bA], nA3[bA]
            tml[p, 9], tml[p, 10], tml[p, 11] = nA1[bB], nA2[bB], nA3[bB]
        in_maps.append({
            "xf": np.ascontiguousarray(x[b0:b0 + 4].reshape(2, 128, TOK)),
            "xh": np.ascontiguousarray(xh_c),
            "xl": np.ascontiguousarray(xl_c),
            "e2h": e2h_d, "e2l": e2l_d,
            "tml": tml, "tmr": tmr,
            "cbt": cbt_d,
        })
    return in_maps


def _run(inputs, codebook, trace=False):
    if "nc" not in _cache:
        _cache["nc"] = _build_module()
    nc = _cache["nc"]
    in_maps = _prep_host(inputs, codebook)
    res = bass_utils.run_bass_kernel_spmd(
        nc, in_maps, core_ids=list(range(NCORES)), trace=trace)
    outs = np.empty((B, D, H, W), np.float32)
    for cid in range(NCORES):
        o = res.results[cid]["out"]              # [2, 128, TOK]
        outs[BPC * cid: BPC * (cid + 1)] = o.reshape(BPC, D, H, W)
    return outs, res


def kernel(inputs, codebook):
    out, _ = _run(inputs, codebook, trace=False)
    return out
